# revision 4
# baseline (speedup 1.0000x reference)
"""Trainium2 Bass kernel for single-head fused-QKV attention.

Reference computation (per batch b):
    qkv = x @ W.T + b          # x:(2048,1024)  W:(3072,1024)  b:(3072,)
    q, k, v = split(qkv, 3)
    out = softmax(q @ k.T) @ v # no 1/sqrt(d) scale, single head

Sharding: 8 cores = (4 batches) x (2 query halves of 1024 tokens each).
Each core projects Q for its half and K/V for its OWN half only; the pair of
cores sharing a batch exchanges K/V halves with a 2-way AllGather.  Gather
output is rank-ordered == original token order, so the SPMD graph reads fixed
offsets and no per-core asymmetry exists anywhere.

All matmuls run as float32r (fp32 with 12-bit mantissa; operand products are
exact in the fp32 accumulator) at full 1 cycle/row TensorE throughput.

Per-core phases (PE work ~15 GFLOP, the 8-way ideal):
  1. Qt = (W_q x_own^T + b_q)       [e,n], e on partitions
  2. Kt_own = (W_k x_own^T + b_k)   [e,m] -> DRAM -> pair AllGather
     (collective + gather-back DMA overlap phase 3)
  3. V_own = (x_own W_v^T + b_v)    [m,dv] -> DRAM -> pair AllGather
     (overlaps the S pass)
  4. S = Qt^T Kt per 128-query tile -> softmax -> unnormalized P spilled to
     DRAM (rowsum reciprocals kept)
  5. P reloaded, PE-transposed to Pt, O = Pt^T V scaled by 1/rowsum
"""

import numpy as np

import concourse.bass as bass
import concourse.tile as tile
from concourse import bacc, mybir
from concourse.bass_utils import run_bass_kernel_spmd
from concourse.masks import make_identity

F32 = mybir.dt.float32
F32R = mybir.dt.float32r
AX = mybir.AxisListType
ALU = mybir.AluOpType
ACT = mybir.ActivationFunctionType

P = 128          # partitions
D = 1024         # hidden
DC = D // P      # 8 contraction chunks
NK = 2048        # keys per batch
NO = 1024        # own tokens per core (queries & own K/V half)
NQT = NO // P    # 8 query tiles
NMT = NK // P    # 16 key tiles
NMC = NK // 512  # 4 key chunks of 512
NVC = D // 512   # 2 dv chunks of 512

N_CORES = 8
PAIRS = [[0, 1], [2, 3], [4, 5], [6, 7]]

# set by test harness to enable NTFF profiling on the SPMD run
TRACE = False
LAST_EXEC_TIME_NS = None


def _round_fp32r(a: np.ndarray) -> np.ndarray:
    """Round fp32 values to the fp32r grid (12-bit mantissa, round-half-up)."""
    bits = np.ascontiguousarray(a, dtype=np.float32).view(np.uint32)
    r = ((bits.astype(np.uint64) + 0x800) & 0xFFFFF000).astype(np.uint32)
    return r.view(np.float32).reshape(a.shape)


def _build():
    nc = bacc.Bacc("TRN2", target_bir_lowering=False, debug=False,
                   num_devices=N_CORES)

    xt_d = nc.dram_tensor("xt", [P, DC, NO], F32R, kind="ExternalInput").ap()
    wqk_d = nc.dram_tensor("wqk", [P, 16, DC, P], F32R, kind="ExternalInput").ap()
    wv_d = nc.dram_tensor("wv", [P, DC, D], F32R, kind="ExternalInput").ap()
    bqk_d = nc.dram_tensor("bqk", [P, 16], F32, kind="ExternalInput").ap()
    bvb_d = nc.dram_tensor("bvb", [P, D], F32, kind="ExternalInput").ap()
    out_d = nc.dram_tensor("out", [NO, D], F32, kind="ExternalOutput").ap()

    with tile.TileContext(nc) as tc:
        with tc.tile_pool(name="consts", bufs=1) as consts, \
             tc.tile_pool(name="stats", bufs=1) as stats, \
             tc.tile_pool(name="pdram", bufs=1, space="DRAM") as pdram:

            bqk_s = consts.tile([P, 16], F32)
            nc.sync.dma_start(bqk_s[:], bqk_d[:])
            bvb_s = consts.tile([P, D], F32)
            nc.sync.dma_start(bvb_s[:], bvb_d[:])
            ident_s = consts.tile([P, P], F32)
            make_identity(nc, ident_s[:])

            recip_s = stats.tile([P, NQT], F32)

            pbuf = pdram.tile([NO, NK], F32)
            kt_own_d = pdram.tile([P, DC, NO], F32R)
            kt_all_d = pdram.tile([2, P, DC, NO], F32R)
            v_own_d = pdram.tile([P, NQT, D], F32R)
            v_all_d = pdram.tile([2, P, NQT, D], F32R)

            with tc.tile_pool(name="v", bufs=1) as v_pool:
                v_s = v_pool.tile([P, NMT, D], F32R)

                with tc.tile_pool(name="qt", bufs=1) as qt_pool:
                    qt_s = qt_pool.tile([P, DC, NO], F32R)

                    with tc.tile_pool(name="xt", bufs=1) as xt_pool:
                        xt_s = xt_pool.tile([P, DC, NO], F32R)
                        nc.sync.dma_start(xt_s[:], xt_d[:])

                        # ---- phase 1: Qt projection (e on part, n free) --
                        with tc.tile_pool(name="wq", bufs=3) as wq_pool, \
                             tc.tile_pool(name="qps", bufs=4,
                                          space="PSUM") as qps:
                            for et in range(DC):
                                wt = wq_pool.tile([P, DC, P], F32R, tag="w")
                                nc.sync.dma_start(wt[:], wqk_d[:, et])
                                for nck in range(NO // 512):
                                    ps = qps.tile([P, 512], F32, tag="ps")
                                    for dc in range(DC):
                                        nc.tensor.matmul(
                                            ps[:], wt[:, dc],
                                            xt_s[:, dc, nck * 512:(nck + 1) * 512],
                                            start=(dc == 0), stop=(dc == DC - 1))
                                    nc.any.tensor_scalar_add(
                                        qt_s[:, et, nck * 512:(nck + 1) * 512],
                                        ps[:], bqk_s[:, et:et + 1])

                        # ---- phase 2: Kt own-half projection + AllGather --
                        with tc.tile_pool(name="wk", bufs=3) as wk_pool, \
                             tc.tile_pool(name="kst", bufs=4) as kst_pool, \
                             tc.tile_pool(name="kps", bufs=4,
                                          space="PSUM") as kps:
                            for et in range(DC):
                                wt = wk_pool.tile([P, DC, P], F32R, tag="w")
                                nc.sync.dma_start(wt[:], wqk_d[:, 8 + et])
                                for mck in range(NO // 512):
                                    ps = kps.tile([P, 512], F32, tag="ps")
                                    for dc in range(DC):
                                        nc.tensor.matmul(
                                            ps[:], wt[:, dc],
                                            xt_s[:, dc, mck * 512:(mck + 1) * 512],
                                            start=(dc == 0), stop=(dc == DC - 1))
                                    st = kst_pool.tile([P, 512], F32R, tag="st")
                                    nc.any.tensor_scalar_add(
                                        st[:], ps[:], bqk_s[:, 8 + et:9 + et])
                                    nc.sync.dma_start(
                                        kt_own_d[:, et, mck * 512:(mck + 1) * 512],
                                        st[:])
                        nc.gpsimd.collective_compute(
                            "AllGather", ALU.bypass, replica_groups=PAIRS,
                            ins=[kt_own_d.opt()], outs=[kt_all_d.opt()])

                        # ---- phase 3: V own-half projection + AllGather --
                        with tc.tile_pool(name="wv", bufs=1) as wv_pool, \
                             tc.tile_pool(name="vst", bufs=4) as vst_pool, \
                             tc.tile_pool(name="vps", bufs=4,
                                          space="PSUM") as vps:
                            wv_s = wv_pool.tile([P, DC, D], F32R)
                            nc.sync.dma_start(wv_s[:], wv_d[:])
                            for mt in range(NQT):
                                for dvc in range(NVC):
                                    ps = vps.tile([P, 512], F32, tag="ps")
                                    for dc in range(DC):
                                        nc.tensor.matmul(
                                            ps[:],
                                            xt_s[:, dc, mt * P:(mt + 1) * P],
                                            wv_s[:, dc, dvc * 512:(dvc + 1) * 512],
                                            start=(dc == 0), stop=(dc == DC - 1))
                                    st = vst_pool.tile([P, 512], F32R, tag="st")
                                    nc.vector.tensor_add(
                                        st[:], ps[:],
                                        bvb_s[:, dvc * 512:(dvc + 1) * 512])
                                    nc.sync.dma_start(
                                        v_own_d[:, mt, dvc * 512:(dvc + 1) * 512],
                                        st[:])
                        nc.gpsimd.collective_compute(
                            "AllGather", ALU.bypass, replica_groups=PAIRS,
                            ins=[v_own_d.opt()], outs=[v_all_d.opt()])

                    # xt freed; load gathered V halves (overlaps S pass)
                    nc.sync.dma_start(v_s[:, 0:NQT, :], v_all_d[0])
                    nc.sync.dma_start(v_s[:, NQT:NMT, :], v_all_d[1])

                    # ---- phase 4: S = Qt^T Kt, softmax, spill P ----------
                    with tc.tile_pool(name="kt", bufs=1) as kt_pool, \
                         tc.tile_pool(name="sps", bufs=8, space="PSUM") as sps, \
                         tc.tile_pool(name="pp", bufs=2) as pp_pool, \
                         tc.tile_pool(name="sm", bufs=2) as sm_pool:
                        kt_s = kt_pool.tile([P, DC, NK], F32R)
                        nc.sync.dma_start(kt_s[:, :, 0:NO], kt_all_d[0])
                        nc.sync.dma_start(kt_s[:, :, NO:NK], kt_all_d[1])

                        for qt in range(NQT):
                            stiles = []
                            for mck in range(NMC):
                                ps = sps.tile([P, 512], F32, tag="s")
                                for ec in range(DC):
                                    nc.tensor.matmul(
                                        ps[:],
                                        qt_s[:, ec, qt * P:(qt + 1) * P],
                                        kt_s[:, ec, mck * 512:(mck + 1) * 512],
                                        start=(ec == 0), stop=(ec == DC - 1))
                                stiles.append(ps)
                            pmax = sm_pool.tile([P, NMC], F32, tag="pmax")
                            for mck in range(NMC):
                                nc.vector.tensor_reduce(
                                    pmax[:, mck:mck + 1], stiles[mck][:],
                                    axis=AX.X, op=ALU.max)
                            nmax = sm_pool.tile([P, 1], F32, tag="nmax")
                            nc.vector.tensor_reduce(
                                nmax[:], pmax[:], axis=AX.X, op=ALU.max,
                                negate=True)
                            psum4 = sm_pool.tile([P, NMC], F32, tag="psum4")
                            ptile = pp_pool.tile([P, NK], F32, tag="p")
                            for mck in range(NMC):
                                nc.scalar.activation(
                                    ptile[:, mck * 512:(mck + 1) * 512],
                                    stiles[mck][:], ACT.Exp,
                                    bias=nmax[:, 0:1], scale=1.0,
                                    accum_out=psum4[:, mck:mck + 1])
                            rsum = sm_pool.tile([P, 1], F32, tag="rsum")
                            nc.vector.tensor_reduce(
                                rsum[:], psum4[:], axis=AX.X, op=ALU.add)
                            nc.vector.reciprocal(recip_s[:, qt:qt + 1], rsum[:])
                            nc.sync.dma_start(
                                pbuf[qt * P:(qt + 1) * P, :], ptile[:])

                # ---- phase 5: Pt = P^T, O = Pt^T V, scale, store --------
                with tc.tile_pool(name="pin", bufs=2) as pin_pool, \
                     tc.tile_pool(name="pts", bufs=2) as pts_pool, \
                     tc.tile_pool(name="tps", bufs=2, space="PSUM") as tps, \
                     tc.tile_pool(name="ops", bufs=4, space="PSUM") as ops_pool, \
                     tc.tile_pool(name="osb", bufs=3) as osb_pool:
                    for qt in range(NQT):
                        pin_t = pin_pool.tile([P, NK], F32, tag="pin")
                        nc.sync.dma_start(pin_t[:], pbuf[qt * P:(qt + 1) * P, :])
                        pts_t = pts_pool.tile([P, NMT, P], F32R, tag="pts")
                        for mt in range(NMT):
                            tp = tps.tile([P, P], F32, tag="tp")
                            nc.tensor.transpose(
                                tp[:], pin_t[:, mt * P:(mt + 1) * P], ident_s[:])
                            nc.any.tensor_copy(out=pts_t[:, mt], in_=tp[:])
                        for dvc in range(NVC):
                            ops = ops_pool.tile([P, 512], F32, tag="o")
                            for mt in range(NMT):
                                nc.tensor.matmul(
                                    ops[:], pts_t[:, mt],
                                    v_s[:, mt, dvc * 512:(dvc + 1) * 512],
                                    start=(mt == 0), stop=(mt == NMT - 1))
                            ot = osb_pool.tile([P, 512], F32, tag="ot")
                            nc.scalar.activation(
                                ot[:], ops[:], ACT.Copy,
                                bias=0.0, scale=recip_s[:, qt:qt + 1])
                            nc.sync.dma_start(
                                out_d[qt * P:(qt + 1) * P,
                                      dvc * 512:(dvc + 1) * 512], ot[:])

    nc.compile()
    return nc


_NC_CACHE = None


def _get_nc():
    global _NC_CACHE
    if _NC_CACHE is None:
        _NC_CACHE = _build()
    return _NC_CACHE


def _prep_inputs(x, W, b):
    """Host-side shard + pack + fp32r-round. Returns in_maps for 8 cores."""
    x = np.asarray(x, dtype=np.float32)
    W = np.asarray(W, dtype=np.float32)
    b = np.asarray(b, dtype=np.float32)

    # W packs (shared across cores)
    wqk = _round_fp32r(
        np.ascontiguousarray(
            W[:2 * D].reshape(16, P, DC, P).transpose(3, 0, 2, 1)))
    wv = _round_fp32r(
        np.ascontiguousarray(W[2 * D:].reshape(D, DC, P).transpose(2, 1, 0)))
    bqk = np.ascontiguousarray(b[:2 * D].reshape(16, P).T)
    bvb = np.ascontiguousarray(np.broadcast_to(b[2 * D:], (P, D)))

    in_maps = []
    for c in range(N_CORES):
        bi, h = divmod(c, 2)
        xo = x[bi][h * NO:(h + 1) * NO]          # own token half
        # xt[p, dc, m] = xo[m, dc*128+p]
        xt = _round_fp32r(np.ascontiguousarray(
            xo.reshape(NO, DC, P).transpose(2, 1, 0)))
        in_maps.append({"xt": xt, "wqk": wqk, "wv": wv, "bqk": bqk,
                        "bvb": bvb})
    return in_maps


def kernel(x, W, b):
    global LAST_EXEC_TIME_NS
    nc = _get_nc()
    in_maps = _prep_inputs(x, W, b)
    res = run_bass_kernel_spmd(nc, in_maps, core_ids=list(range(N_CORES)),
                               trace=TRACE)
    LAST_EXEC_TIME_NS = res.exec_time_ns
    out = np.empty((4, NK, D), dtype=np.float32)
    for c in range(N_CORES):
        bi, h = divmod(c, 2)
        out[bi, h * NO:(h + 1) * NO, :] = res.results[c]["out"]
    return out


# revision 17
# speedup vs baseline: 1.3726x; 1.3726x over previous
"""Trainium2 Bass kernel for single-head fused-QKV attention.

Reference computation (per batch b):
    qkv = x @ W.T + b          # x:(2048,1024)  W:(3072,1024)  b:(3072,)
    q, k, v = split(qkv, 3)
    out = softmax(q @ k.T) @ v # no 1/sqrt(d) scale, single head

Sharding: 8 cores = (4 batches) x (2 query halves of 1024 tokens each).
Each core projects Q for its 1024 queries and K/V for the full 2048-token
sequence of its batch (K/V projection duplicated within the batch pair --
measured 2-rank collectives run at only ~36 GB/s and peer SBUF DMA is not
functional in this runtime, so staying comm-free is fastest).  Host-side,
the token axis is rotated per-core so each core's query half occupies
tokens [0,1024) -- softmax(QK^T)V is invariant to a consistent permutation
of the key/value axis, so the graph stays SPMD.

All matmuls run as float32r (fp32 with 12-bit mantissa; operand products
are exact in the fp32 accumulator) at full 1 cycle/row TensorE throughput.

Per-core phases:
  1. Qt = (W_q x_q^T + b_q)   [e,n] layout, e on partitions (only needs the
     first half of xt, so compute starts as soon as the first 4MB DMA lands)
  2. Kt = (W_k x^T + b_k)     [e,m] layout, all 2048 keys
  3. S = Qt^T Kt per 128-query tile -> softmax -> unnormalized P (fp32r)
     spilled to DRAM (rowsum reciprocals kept); frees Kt/Qt SBUF
  4. V = x W_v^T + b_v        [m,dv] layout (W_v chunk 0 prefetched into
     right-side SBUF during phase 3; bias added on DVE during PSUM eviction)
  5. P reloaded, PE-transposed (fp32r, 4 tiles packed per PSUM bank),
     O = Pt^T V scaled by 1/rowsum
"""

import numpy as np

import concourse.bass as bass
import concourse.tile as tile
from concourse import bacc, mybir
from concourse.bass_utils import run_bass_kernel_spmd
from concourse.masks import make_identity

F32 = mybir.dt.float32
F32R = mybir.dt.float32r
AX = mybir.AxisListType
ALU = mybir.AluOpType
ACT = mybir.ActivationFunctionType

P = 128          # partitions
D = 1024         # hidden
DC = D // P      # 8 contraction chunks
NK = 2048        # keys per batch
NQ = 1024        # queries per core
NQT = NQ // P    # 8 query tiles
NMT = NK // P    # 16 key tiles
NMC = NK // 512  # 4 key chunks of 512
NVC = D // 512   # 2 dv chunks of 512

N_CORES = 8

# set by test harness to enable NTFF profiling on the SPMD run
TRACE = False
LAST_EXEC_TIME_NS = None


def _round_fp32r(a: np.ndarray) -> np.ndarray:
    """Round fp32 values to the fp32r grid (12-bit mantissa, round-half-up)."""
    bits = np.ascontiguousarray(a, dtype=np.float32).view(np.uint32)
    r = ((bits.astype(np.uint64) + 0x800) & 0xFFFFF000).astype(np.uint32)
    return r.view(np.float32).reshape(a.shape)


def _phase_qk(nc, tc, w_d_col0, xt_s, out_s, bqk_s, bcol0, n_cols, pname):
    """Shared Q/K projection phase: out_s[:, et, :] = W_et x^T + b."""
    with tc.tile_pool(name=f"w{pname}", bufs=3) as w_pool, \
         tc.tile_pool(name=f"{pname}ps", bufs=4, space="PSUM") as psp:
        for et in range(DC):
            wt = w_pool.tile([P, DC, P], F32R, tag="w")
            nc.sync.dma_start(wt[:], w_d_col0[:, bcol0 + et])
            for ck in range(n_cols // 512):
                ps = psp.tile([P, 512], F32, tag="ps")
                for dc in range(DC):
                    nc.tensor.matmul(
                        ps[:], wt[:, dc],
                        xt_s[:, dc, ck * 512:(ck + 1) * 512],
                        start=(dc == 0), stop=(dc == DC - 1))
                nc.vector.tensor_scalar_add(
                    out_s[:, et, ck * 512:(ck + 1) * 512], ps[:],
                    bqk_s[:, bcol0 + et:bcol0 + et + 1])


def _build():
    nc = bacc.Bacc("TRN2", target_bir_lowering=False, debug=False,
                   num_devices=N_CORES)

    xt_d = nc.dram_tensor("xt", [P, DC, NK], F32R, kind="ExternalInput").ap()
    wqk_d = nc.dram_tensor("wqk", [P, 16, DC, P], F32R, kind="ExternalInput").ap()
    wv_d = nc.dram_tensor("wv", [P, DC, D], F32R, kind="ExternalInput").ap()
    bqk_d = nc.dram_tensor("bqk", [P, 16], F32, kind="ExternalInput").ap()
    bvb_d = nc.dram_tensor("bvb", [P, D], F32, kind="ExternalInput").ap()
    out_d = nc.dram_tensor("out", [NQ, D], F32, kind="ExternalOutput").ap()

    with tile.TileContext(nc) as tc:
        with tc.tile_pool(name="consts", bufs=1) as consts, \
             tc.tile_pool(name="stats", bufs=1) as stats, \
             tc.tile_pool(name="pdram", bufs=1, space="DRAM") as pdram:

            bqk_s = consts.tile([P, 16], F32)
            nc.sync.dma_start(bqk_s[:], bqk_d[:])
            bvb_s = consts.tile([P, D], F32)
            nc.sync.dma_start(bvb_s[:], bvb_d[:])
            ident_s = consts.tile([P, P], F32R)
            with tc.tile_pool(name="identf", bufs=1) as identf_pool:
                ident_f = identf_pool.tile([P, P], F32)
                make_identity(nc, ident_f[:])
                nc.vector.tensor_copy(out=ident_s[:], in_=ident_f[:])

            recip_s = stats.tile([P, NQT], F32)
            pbuf = pdram.tile([NQ, NK], F32R)

            with tc.tile_pool(name="xt", bufs=1) as xt_pool:
                xt_s = xt_pool.tile([P, DC, NK], F32R)
                # query-half columns first so phase 1 starts early
                nc.sync.dma_start(xt_s[:, :, 0:NQ], xt_d[:, :, 0:NQ])
                nc.sync.dma_start(xt_s[:, :, NQ:NK], xt_d[:, :, NQ:NK])

                with tc.tile_pool(name="wv", bufs=1, side="right") as wv_pool:
                    wv_c = [None] * NVC
                    wv_c0 = wv_pool.tile([P, DC, 512], F32R, tag="wv")
                    wv_c[0] = wv_c0
                    nc.sync.dma_start(wv_c0[:], wv_d[:, :, 0:512])

                    with tc.tile_pool(name="qt", bufs=1) as qt_pool, \
                         tc.tile_pool(name="kt", bufs=1) as kt_pool:
                        qt_s = qt_pool.tile([P, DC, NQ], F32R)
                        kt_s = kt_pool.tile([P, DC, NK], F32R)

                        # phase 1: Qt projection (e on partitions, n free)
                        _phase_qk(nc, tc, wqk_d, xt_s, qt_s, bqk_s, 0, NQ, "q")
                        # phase 2: Kt projection (all 2048 keys)
                        _phase_qk(nc, tc, wqk_d, xt_s, kt_s, bqk_s, 8, NK, "k")

                        # phase 3: S = Qt^T Kt, softmax, spill P
                        with tc.tile_pool(name="sps", bufs=8,
                                          space="PSUM") as sps, \
                             tc.tile_pool(name="pp", bufs=2) as pp_pool, \
                             tc.tile_pool(name="sm", bufs=2) as sm_pool:
                            for qt in range(NQT):
                                stiles = []
                                for mck in range(NMC):
                                    ps = sps.tile([P, 512], F32, tag="s")
                                    for ec in range(DC):
                                        nc.tensor.matmul(
                                            ps[:],
                                            qt_s[:, ec, qt * P:(qt + 1) * P],
                                            kt_s[:, ec,
                                                 mck * 512:(mck + 1) * 512],
                                            start=(ec == 0),
                                            stop=(ec == DC - 1))
                                    stiles.append(ps)
                                # packed stats: [0:4]=chunk maxes, [4:8]=
                                # chunk exp-sums, [8]=neg row max, [9]=row sum
                                sm = sm_pool.tile([P, 12], F32, tag="sm")
                                for mck in range(NMC):
                                    nc.vector.tensor_reduce(
                                        sm[:, mck:mck + 1], stiles[mck][:],
                                        axis=AX.X, op=ALU.max)
                                nc.vector.tensor_reduce(
                                    sm[:, 8:9], sm[:, 0:4], axis=AX.X,
                                    op=ALU.max, negate=True)
                                ptile = pp_pool.tile([P, NK], F32R, tag="p")
                                for mck in range(NMC):
                                    nc.scalar.activation(
                                        ptile[:, mck * 512:(mck + 1) * 512],
                                        stiles[mck][:], ACT.Exp,
                                        bias=sm[:, 8:9], scale=1.0,
                                        accum_out=sm[:, 4 + mck:5 + mck])
                                nc.vector.tensor_reduce(
                                    sm[:, 9:10], sm[:, 4:8], axis=AX.X,
                                    op=ALU.add)
                                nc.vector.reciprocal(recip_s[:, qt:qt + 1],
                                                     sm[:, 9:10])
                                nc.sync.dma_start(
                                    pbuf[qt * P:(qt + 1) * P, :], ptile[:])

                    # qt/kt freed; phase 4: V projection ([m, dv] layout)
                    with tc.tile_pool(name="v", bufs=1) as v_pool:
                        v_s = v_pool.tile([P, NMT, D], F32R)
                        with tc.tile_pool(name="vps", bufs=4,
                                          space="PSUM") as vps:
                            for dvc in range(NVC):
                                if wv_c[dvc] is None:
                                    wv_cn = wv_pool.tile([P, DC, 512], F32R,
                                                         tag="wv")
                                    wv_c[dvc] = wv_cn
                                    nc.sync.dma_start(
                                        wv_cn[:],
                                        wv_d[:, :, dvc * 512:(dvc + 1) * 512])
                                for mt in range(NMT):
                                    ps = vps.tile([P, 512], F32, tag="ps")
                                    for dc in range(DC):
                                        nc.tensor.matmul(
                                            ps[:],
                                            xt_s[:, dc, mt * P:(mt + 1) * P],
                                            wv_c[dvc][:, dc],
                                            start=(dc == 0),
                                            stop=(dc == DC - 1))
                                    nc.vector.tensor_add(
                                        v_s[:, mt, dvc * 512:(dvc + 1) * 512],
                                        ps[:],
                                        bvb_s[:, dvc * 512:(dvc + 1) * 512])

                        # phase 5: Pt = P^T, O = Pt^T V, scale, store
                        with tc.tile_pool(name="pin", bufs=2) as pin_pool, \
                             tc.tile_pool(name="pts", bufs=2) as pts_pool, \
                             tc.tile_pool(name="tps", bufs=2,
                                          space="PSUM") as tps, \
                             tc.tile_pool(name="ops", bufs=4,
                                          space="PSUM") as ops_pool, \
                             tc.tile_pool(name="osb", bufs=3) as osb_pool:
                            for qt in range(NQT):
                                pin_t = pin_pool.tile([P, NK], F32R, tag="pin")
                                nc.sync.dma_start(
                                    pin_t[:], pbuf[qt * P:(qt + 1) * P, :])
                                pts_t = pts_pool.tile([P, NMT, P], F32R,
                                                      tag="pts")
                                # 4 transposed 128x128 tiles per PSUM bank
                                for grp in range(NMT // 4):
                                    tp = tps.tile([P, 4, P], F32R, tag="tp")
                                    for j in range(4):
                                        mt = grp * 4 + j
                                        nc.tensor.transpose(
                                            tp[:, j],
                                            pin_t[:, mt * P:(mt + 1) * P],
                                            ident_s[:])
                                    nc.any.tensor_copy(
                                        out=pts_t[:, grp * 4:(grp + 1) * 4],
                                        in_=tp[:])
                                for dvc in range(NVC):
                                    ops = ops_pool.tile([P, 512], F32, tag="o")
                                    for mt in range(NMT):
                                        nc.tensor.matmul(
                                            ops[:], pts_t[:, mt],
                                            v_s[:, mt,
                                                dvc * 512:(dvc + 1) * 512],
                                            start=(mt == 0),
                                            stop=(mt == NMT - 1))
                                    ot = osb_pool.tile([P, 512], F32, tag="ot")
                                    nc.scalar.activation(
                                        ot[:], ops[:], ACT.Copy,
                                        bias=0.0, scale=recip_s[:, qt:qt + 1])
                                    nc.sync.dma_start(
                                        out_d[qt * P:(qt + 1) * P,
                                              dvc * 512:(dvc + 1) * 512],
                                        ot[:])

    nc.compile()
    return nc


_NC_CACHE = None


def _get_nc():
    global _NC_CACHE
    if _NC_CACHE is None:
        _NC_CACHE = _build()
    return _NC_CACHE


def _prep_inputs(x, W, b):
    """Host-side shard + pack + fp32r-round. Returns in_maps for 8 cores."""
    x = np.asarray(x, dtype=np.float32)
    W = np.asarray(W, dtype=np.float32)
    b = np.asarray(b, dtype=np.float32)

    # W packs (shared across cores)
    wqk = _round_fp32r(
        np.ascontiguousarray(
            W[:2 * D].reshape(16, P, DC, P).transpose(3, 0, 2, 1)))
    wv = _round_fp32r(
        np.ascontiguousarray(W[2 * D:].reshape(D, DC, P).transpose(2, 1, 0)))
    bqk = np.ascontiguousarray(b[:2 * D].reshape(16, P).T)
    bvb = np.ascontiguousarray(np.broadcast_to(b[2 * D:], (P, D)))

    in_maps = []
    for c in range(N_CORES):
        bi, h = divmod(c, 2)
        xb = x[bi]
        if h:
            xb = np.concatenate([xb[NQ:], xb[:NQ]], axis=0)
        # xt[p, dc, m] = xb[m, dc*128+p]
        xt = _round_fp32r(np.ascontiguousarray(
            xb.reshape(NK, DC, P).transpose(2, 1, 0)))
        in_maps.append({"xt": xt, "wqk": wqk, "wv": wv, "bqk": bqk,
                        "bvb": bvb})
    return in_maps


def kernel(x, W, b):
    global LAST_EXEC_TIME_NS
    nc = _get_nc()
    in_maps = _prep_inputs(x, W, b)
    res = run_bass_kernel_spmd(nc, in_maps, core_ids=list(range(N_CORES)),
                               trace=TRACE)
    LAST_EXEC_TIME_NS = res.exec_time_ns
    out = np.empty((4, NK, D), dtype=np.float32)
    for c in range(N_CORES):
        bi, h = divmod(c, 2)
        out[bi, h * NQ:(h + 1) * NQ, :] = res.results[c]["out"]
    return out


# revision 18
# speedup vs baseline: 1.4806x; 1.0787x over previous
"""Trainium2 Bass kernel for single-head fused-QKV attention.

Reference computation (per batch b):
    qkv = x @ W.T + b          # x:(2048,1024)  W:(3072,1024)  b:(3072,)
    q, k, v = split(qkv, 3)
    out = softmax(q @ k.T) @ v # no 1/sqrt(d) scale, single head

Sharding: 8 cores = (4 batches) x (2 query halves of 1024 tokens each).
Each core projects Q for its 1024 queries and K/V for the full 2048-token
sequence of its batch (K/V projection duplicated within the batch pair --
measured 2-rank collectives run at only ~36 GB/s and peer SBUF DMA is not
functional in this runtime, so staying comm-free is fastest).  Host-side,
the token axis is rotated per-core so each core's query half occupies
tokens [0,1024) -- softmax(QK^T)V is invariant to a consistent permutation
of the key/value axis, so the graph stays SPMD.

All matmuls run as float32r (fp32 with 12-bit mantissa; operand products
are exact in the fp32 accumulator) at full 1 cycle/row TensorE throughput.

Per-core phases:
  1. Qt = (W_q x_q^T + b_q)   [e,n] layout, e on partitions (only needs the
     first half of xt, so compute starts as soon as the first 4MB DMA lands)
  2. Kt = (W_k x^T + b_k)     [e,m] layout, all 2048 keys
  3. S = Qt^T Kt per 128-query tile -> softmax -> unnormalized P (fp32r)
     spilled to DRAM (rowsum reciprocals kept); frees Kt/Qt SBUF
  4. V = x W_v^T + b_v        [m,dv] layout (W_v chunk 0 prefetched into
     right-side SBUF during phase 3; bias added on DVE during PSUM eviction)
  5. P reloaded, PE-transposed (fp32r, 4 tiles packed per PSUM bank),
     O = Pt^T V scaled by 1/rowsum
"""

import numpy as np

import concourse.bass as bass
import concourse.tile as tile
from concourse import bacc, mybir
from concourse.bass_utils import run_bass_kernel_spmd
from concourse.masks import make_identity

F32 = mybir.dt.float32
F32R = mybir.dt.float32r
AX = mybir.AxisListType
ALU = mybir.AluOpType
ACT = mybir.ActivationFunctionType

P = 128          # partitions
D = 1024         # hidden
DC = D // P      # 8 contraction chunks
NK = 2048        # keys per batch
NQ = 1024        # queries per core
NQT = NQ // P    # 8 query tiles
NMT = NK // P    # 16 key tiles
NMC = NK // 512  # 4 key chunks of 512
NVC = D // 512   # 2 dv chunks of 512

N_CORES = 8

# set by test harness to enable NTFF profiling on the SPMD run
TRACE = False
LAST_EXEC_TIME_NS = None


def _round_fp32r(a: np.ndarray) -> np.ndarray:
    """Round fp32 values to the fp32r grid (12-bit mantissa, round-half-up)."""
    bits = np.ascontiguousarray(a, dtype=np.float32).view(np.uint32)
    r = ((bits.astype(np.uint64) + 0x800) & 0xFFFFF000).astype(np.uint32)
    return r.view(np.float32).reshape(a.shape)


def _phase_qk(nc, tc, w_d_col0, xt_s, out_s, bqk_s, bcol0, n_cols, pname):
    """Shared Q/K projection phase: out_s[:, et, :] = W_et x^T + b."""
    with tc.tile_pool(name=f"w{pname}", bufs=3) as w_pool, \
         tc.tile_pool(name=f"{pname}ps", bufs=4, space="PSUM") as psp:
        for et in range(DC):
            wt = w_pool.tile([P, DC, P], F32R, tag="w")
            nc.scalar.dma_start(wt[:], w_d_col0[:, bcol0 + et])
            for ck in range(n_cols // 512):
                ps = psp.tile([P, 512], F32, tag="ps")
                for dc in range(DC):
                    nc.tensor.matmul(
                        ps[:], wt[:, dc],
                        xt_s[:, dc, ck * 512:(ck + 1) * 512],
                        start=(dc == 0), stop=(dc == DC - 1))
                nc.vector.tensor_scalar_add(
                    out_s[:, et, ck * 512:(ck + 1) * 512], ps[:],
                    bqk_s[:, bcol0 + et:bcol0 + et + 1])


def _build():
    nc = bacc.Bacc("TRN2", target_bir_lowering=False, debug=False,
                   num_devices=N_CORES)

    xt_d = nc.dram_tensor("xt", [P, DC, NK], F32R, kind="ExternalInput").ap()
    wqk_d = nc.dram_tensor("wqk", [P, 16, DC, P], F32R, kind="ExternalInput").ap()
    wv_d = nc.dram_tensor("wv", [P, DC, D], F32R, kind="ExternalInput").ap()
    bqk_d = nc.dram_tensor("bqk", [P, 16], F32, kind="ExternalInput").ap()
    bvb_d = nc.dram_tensor("bvb", [P, D], F32, kind="ExternalInput").ap()
    out_d = nc.dram_tensor("out", [NQ, D], F32, kind="ExternalOutput").ap()

    with tile.TileContext(nc) as tc:
        with tc.tile_pool(name="consts", bufs=1) as consts, \
             tc.tile_pool(name="stats", bufs=1) as stats, \
             tc.tile_pool(name="pdram", bufs=1, space="DRAM") as pdram:

            bqk_s = consts.tile([P, 16], F32)
            nc.scalar.dma_start(bqk_s[:], bqk_d[:])
            bvb_s = consts.tile([P, D], F32)
            nc.scalar.dma_start(bvb_s[:], bvb_d[:])
            ident_s = consts.tile([P, P], F32R)
            with tc.tile_pool(name="identf", bufs=1) as identf_pool:
                ident_f = identf_pool.tile([P, P], F32)
                make_identity(nc, ident_f[:])
                nc.vector.tensor_copy(out=ident_s[:], in_=ident_f[:])

            recip_s = stats.tile([P, NQT], F32)
            pbuf = pdram.tile([NQ, NK], F32R)

            with tc.tile_pool(name="xt", bufs=1) as xt_pool:
                xt_s = xt_pool.tile([P, DC, NK], F32R)
                # query-half columns first so phase 1 starts early
                nc.sync.dma_start(xt_s[:, :, 0:NQ], xt_d[:, :, 0:NQ])
                nc.sync.dma_start(xt_s[:, :, NQ:NK], xt_d[:, :, NQ:NK])

                with tc.tile_pool(name="wv", bufs=1, side="right") as wv_pool:
                    wv_c = [None] * NVC
                    wv_c0 = wv_pool.tile([P, DC, 512], F32R, tag="wv")
                    wv_c[0] = wv_c0
                    nc.sync.dma_start(wv_c0[:], wv_d[:, :, 0:512])

                    with tc.tile_pool(name="qt", bufs=1) as qt_pool, \
                         tc.tile_pool(name="kt", bufs=1) as kt_pool:
                        qt_s = qt_pool.tile([P, DC, NQ], F32R)
                        kt_s = kt_pool.tile([P, DC, NK], F32R)

                        # phase 1: Qt projection (e on partitions, n free)
                        _phase_qk(nc, tc, wqk_d, xt_s, qt_s, bqk_s, 0, NQ, "q")
                        # phase 2: Kt projection (all 2048 keys)
                        _phase_qk(nc, tc, wqk_d, xt_s, kt_s, bqk_s, 8, NK, "k")

                        # phase 3: S = Qt^T Kt, softmax, spill P
                        with tc.tile_pool(name="sps", bufs=6,
                                          space="PSUM") as sps, \
                             tc.tile_pool(name="pp", bufs=2) as pp_pool, \
                             tc.tile_pool(name="sm", bufs=2) as sm_pool:
                            for qt in range(NQT):
                                stiles = []
                                for mck in range(NMC):
                                    ps = sps.tile([P, 512], F32, tag="s")
                                    for ec in range(DC):
                                        nc.tensor.matmul(
                                            ps[:],
                                            qt_s[:, ec, qt * P:(qt + 1) * P],
                                            kt_s[:, ec,
                                                 mck * 512:(mck + 1) * 512],
                                            start=(ec == 0),
                                            stop=(ec == DC - 1))
                                    stiles.append(ps)
                                # packed stats: [0:4]=chunk maxes, [4:8]=
                                # chunk exp-sums, [8]=neg row max, [9]=row sum
                                sm = sm_pool.tile([P, 12], F32, tag="sm")
                                for mck in range(NMC):
                                    nc.vector.tensor_reduce(
                                        sm[:, mck:mck + 1], stiles[mck][:],
                                        axis=AX.X, op=ALU.max)
                                nc.vector.tensor_reduce(
                                    sm[:, 8:9], sm[:, 0:4], axis=AX.X,
                                    op=ALU.max, negate=True)
                                ptile = pp_pool.tile([P, NK], F32R, tag="p")
                                for mck in range(NMC):
                                    nc.scalar.activation(
                                        ptile[:, mck * 512:(mck + 1) * 512],
                                        stiles[mck][:], ACT.Exp,
                                        bias=sm[:, 8:9], scale=1.0,
                                        accum_out=sm[:, 4 + mck:5 + mck])
                                nc.vector.tensor_reduce(
                                    sm[:, 9:10], sm[:, 4:8], axis=AX.X,
                                    op=ALU.add)
                                nc.vector.reciprocal(recip_s[:, qt:qt + 1],
                                                     sm[:, 9:10])
                                nc.sync.dma_start(
                                    pbuf[qt * P:(qt + 1) * P, :], ptile[:])

                    # qt/kt freed; phase 4: V projection ([m, dv] layout)
                    with tc.tile_pool(name="v", bufs=1) as v_pool:
                        v_s = v_pool.tile([P, NMT, D], F32R)
                        with tc.tile_pool(name="vps", bufs=2,
                                          space="PSUM",
                                          side="right") as vps:
                            for dvc in range(NVC):
                                if wv_c[dvc] is None:
                                    wv_cn = wv_pool.tile([P, DC, 512], F32R,
                                                         tag="wv")
                                    wv_c[dvc] = wv_cn
                                    nc.sync.dma_start(
                                        wv_cn[:],
                                        wv_d[:, :, dvc * 512:(dvc + 1) * 512])
                                for mt in range(NMT):
                                    ps = vps.tile([P, 512], F32, tag="ps")
                                    for dc in range(DC):
                                        nc.tensor.matmul(
                                            ps[:],
                                            xt_s[:, dc, mt * P:(mt + 1) * P],
                                            wv_c[dvc][:, dc],
                                            start=(dc == 0),
                                            stop=(dc == DC - 1))
                                    nc.vector.tensor_add(
                                        v_s[:, mt, dvc * 512:(dvc + 1) * 512],
                                        ps[:],
                                        bvb_s[:, dvc * 512:(dvc + 1) * 512])

                        # phase 5: Pt = P^T, O = Pt^T V, scale, store
                        with tc.tile_pool(name="pin", bufs=2) as pin_pool, \
                             tc.tile_pool(name="pts", bufs=2) as pts_pool, \
                             tc.tile_pool(name="tps", bufs=2,
                                          space="PSUM") as tps, \
                             tc.tile_pool(name="ops", bufs=4,
                                          space="PSUM") as ops_pool, \
                             tc.tile_pool(name="osb", bufs=3) as osb_pool:
                            for qt in range(NQT):
                                pin_t = pin_pool.tile([P, NK], F32R, tag="pin")
                                nc.sync.dma_start(
                                    pin_t[:], pbuf[qt * P:(qt + 1) * P, :])
                                pts_t = pts_pool.tile([P, NMT, P], F32R,
                                                      tag="pts")
                                # 4 transposed 128x128 tiles per PSUM bank
                                for grp in range(NMT // 4):
                                    tp = tps.tile([P, 4, P], F32R, tag="tp")
                                    for j in range(4):
                                        mt = grp * 4 + j
                                        nc.tensor.transpose(
                                            tp[:, j],
                                            pin_t[:, mt * P:(mt + 1) * P],
                                            ident_s[:])
                                    nc.any.tensor_copy(
                                        out=pts_t[:, grp * 4:(grp + 1) * 4],
                                        in_=tp[:])
                                for dvc in range(NVC):
                                    ops = ops_pool.tile([P, 512], F32, tag="o")
                                    for mt in range(NMT):
                                        nc.tensor.matmul(
                                            ops[:], pts_t[:, mt],
                                            v_s[:, mt,
                                                dvc * 512:(dvc + 1) * 512],
                                            start=(mt == 0),
                                            stop=(mt == NMT - 1))
                                    ot = osb_pool.tile([P, 512], F32, tag="ot")
                                    nc.scalar.activation(
                                        ot[:], ops[:], ACT.Copy,
                                        bias=0.0, scale=recip_s[:, qt:qt + 1])
                                    nc.sync.dma_start(
                                        out_d[qt * P:(qt + 1) * P,
                                              dvc * 512:(dvc + 1) * 512],
                                        ot[:])

    nc.compile()
    return nc


_NC_CACHE = None


def _get_nc():
    global _NC_CACHE
    if _NC_CACHE is None:
        _NC_CACHE = _build()
    return _NC_CACHE


def _prep_inputs(x, W, b):
    """Host-side shard + pack + fp32r-round. Returns in_maps for 8 cores."""
    x = np.asarray(x, dtype=np.float32)
    W = np.asarray(W, dtype=np.float32)
    b = np.asarray(b, dtype=np.float32)

    # W packs (shared across cores)
    wqk = _round_fp32r(
        np.ascontiguousarray(
            W[:2 * D].reshape(16, P, DC, P).transpose(3, 0, 2, 1)))
    wv = _round_fp32r(
        np.ascontiguousarray(W[2 * D:].reshape(D, DC, P).transpose(2, 1, 0)))
    bqk = np.ascontiguousarray(b[:2 * D].reshape(16, P).T)
    bvb = np.ascontiguousarray(np.broadcast_to(b[2 * D:], (P, D)))

    in_maps = []
    for c in range(N_CORES):
        bi, h = divmod(c, 2)
        xb = x[bi]
        if h:
            xb = np.concatenate([xb[NQ:], xb[:NQ]], axis=0)
        # xt[p, dc, m] = xb[m, dc*128+p]
        xt = _round_fp32r(np.ascontiguousarray(
            xb.reshape(NK, DC, P).transpose(2, 1, 0)))
        in_maps.append({"xt": xt, "wqk": wqk, "wv": wv, "bqk": bqk,
                        "bvb": bvb})
    return in_maps


def kernel(x, W, b):
    global LAST_EXEC_TIME_NS
    nc = _get_nc()
    in_maps = _prep_inputs(x, W, b)
    res = run_bass_kernel_spmd(nc, in_maps, core_ids=list(range(N_CORES)),
                               trace=TRACE)
    LAST_EXEC_TIME_NS = res.exec_time_ns
    out = np.empty((4, NK, D), dtype=np.float32)
    for c in range(N_CORES):
        bi, h = divmod(c, 2)
        out[bi, h * NQ:(h + 1) * NQ, :] = res.results[c]["out"]
    return out
